# revision 15
# baseline (speedup 1.0000x reference)
"""Linformer self-attention on 8 Trainium2 NeuronCores.

Problem (hardcoded shapes): x [4,4096,1024] f32; per batch:
  q = scale*(x@Wq); kv = x@Wkv; keys/values compressed 4096->256 via
  proj_k/proj_v; 16-head attention (dh=64, k=256); out @ Wproj + bproj.

Sharding: 8 cores = 4 batches x 2 head-groups (8 heads / 512 cols each).
Each core computes a partial [4096,1024] output (Wproj row-split); host
sums the pair and adds bias.

Per-core dataflow (all matmuls use out = lhsT.T @ rhs, K<=128 partitions):
  A : xcxvT[1024,512] = x.T @ [proj_k|proj_v]          (contract n, x natural lhsT)
  A2: kprojT[512,256] = Wk_g.T @ xcT ; vproj[256,512] = xvT.T @ Wv_g
  B : qT[512,4096] = Wq_g.T @ x.T    (x transposed on PE per [128,128] tile)
  C : per head, per n-block: AT[256,n] = kprojT_h.T @ qT_h  -> exp (ACT, no
      max-subtract: |scores| < ~6) -> column sums via ones-matmul ->
      transpose-stats -> reciprocal
  D : O[n,64] = PexpT.T @ vproj_h, normalized by recip at PSUM eviction
  E : out[n,1024] = O.T-transpose @ Wproj_g
"""

import os
import numpy as np

import concourse.bass as bass
import concourse.mybir as mybir
import concourse.tile as tile
from concourse import bacc
from concourse.bass_utils import run_bass_kernel_spmd
from concourse.masks import make_identity

P = 128
N, D, K, DG, DH = 4096, 1024, 256, 512, 64
NCHUNKS = N // P          # 32 chunks of 128 rows
NB = 8                    # n-blocks of 512
HL = 8                    # heads per core
F32 = mybir.dt.float32

# matmul operand dtype: float32r = fp32 data at ~bf16 PE rate for N>=256
MMDT_NAME = os.environ.get("LINF_MMDT", "bfloat16")
MMDT = getattr(mybir.dt, MMDT_NAME)

_cache = {}


def build_nc():
    nc = bacc.Bacc(None, target_bir_lowering=False, debug=False)

    x_d = nc.dram_tensor("x", [N, D], MMDT, kind="ExternalInput")
    pkv_d = nc.dram_tensor("projkv", [N, 2 * K], MMDT, kind="ExternalInput")
    wq_d = nc.dram_tensor("wq", [D, DG], MMDT, kind="ExternalInput")
    wk_d = nc.dram_tensor("wk", [D, DG], MMDT, kind="ExternalInput")
    wv_d = nc.dram_tensor("wv", [D, DG], MMDT, kind="ExternalInput")
    wp_d = nc.dram_tensor("wproj", [DG, D], MMDT, kind="ExternalInput")
    out_d = nc.dram_tensor("out", [N, D], F32, kind="ExternalOutput")

    with tile.TileContext(nc) as tc:
        from contextlib import ExitStack
        with ExitStack() as ctx:
            res = ctx.enter_context(tc.tile_pool(name="res", bufs=1))
            id_mm = res.tile([P, P], MMDT, tag="id_mm")
            make_identity(nc, id_mm[:])
            id_f32 = res.tile([P, P], F32, tag="id_f32")
            make_identity(nc, id_f32[:])
            # sel[:, m*4:m*4+4] is a [128,4] matrix whose column m is all
            # ones: used to matmul-accumulate per-head exp-sums into row m of
            # a [4, 512] PSUM tile (engine partition offsets must be 0/32/64,
            # so rows are written via matmul selectors, not offset copies).
            sel_sb = res.tile([P, 16], MMDT, tag="sel")
            nc.any.memset(sel_sb[:], 0.0)
            for m in range(4):
                nc.any.memset(sel_sb[:, m * 4 + m: m * 4 + m + 1], 1.0)

            wproj_sb = res.tile([P, 4 * D], MMDT, tag="wproj")
            for jc in range(4):
                nc.sync.dma_start(out=wproj_sb[:, jc * D:(jc + 1) * D],
                                  in_=wp_d[jc * P:(jc + 1) * P, :])
            kprojT_sb = res.tile([P, 4 * K], MMDT, tag="kprojT")
            vproj_sb = res.tile([P, 2 * DG], MMDT, tag="vproj")
            qT_sb = res.tile([P, 4 * N], MMDT, tag="qT")
            sums_sb0 = res.tile([4, N], F32, tag="sums0")
            sums_sb1 = res.tile([4, N], F32, tag="sums1")
            sums_halves = [sums_sb0, sums_sb1]
            recips_sb = res.tile([P, NCHUNKS * HL], F32, tag="recips")

            # ---------------- Phase A + A2 ----------------
            with ExitStack() as actx:
                wkvp = actx.enter_context(tc.tile_pool(name="wkv", bufs=1))
                xcxvp = actx.enter_context(tc.tile_pool(name="xcxv", bufs=1))
                xin = actx.enter_context(tc.tile_pool(name="xin", bufs=3))

                wk_sb = wkvp.tile([P, 8 * DG], MMDT, tag="wk")
                wv_sb = wkvp.tile([P, 8 * DG], MMDT, tag="wv")
                for dd in range(8):
                    nc.sync.dma_start(out=wk_sb[:, dd * DG:(dd + 1) * DG],
                                      in_=wk_d[dd * P:(dd + 1) * P, :])
                    nc.sync.dma_start(out=wv_sb[:, dd * DG:(dd + 1) * DG],
                                      in_=wv_d[dd * P:(dd + 1) * P, :])
                xcxv_sb = xcxvp.tile([P, 8 * 2 * K], MMDT, tag="xcxv")

                with tc.tile_pool(name="pa", bufs=1, space="PSUM") as pa:
                    accs = [pa.tile([P, 2 * K], F32, tag=f"pa{dd}", name=f"pa{dd}") for dd in range(8)]
                    for nn in range(NCHUNKS):
                        x_t = xin.tile([P, D], MMDT, tag="x_t")
                        kv_t = xin.tile([P, 2 * K], MMDT, tag="kv_t")
                        nc.sync.dma_start(out=x_t[:], in_=x_d[nn * P:(nn + 1) * P, :])
                        nc.sync.dma_start(out=kv_t[:], in_=pkv_d[nn * P:(nn + 1) * P, :])
                        for dd in range(8):
                            nc.tensor.matmul(accs[dd][:],
                                             lhsT=x_t[:, dd * P:(dd + 1) * P],
                                             rhs=kv_t[:],
                                             start=(nn == 0), stop=(nn == NCHUNKS - 1))
                    for dd in range(8):
                        nc.scalar.copy(out=xcxv_sb[:, dd * 2 * K:(dd + 1) * 2 * K],
                                       in_=accs[dd][:])

                pa2 = actx.enter_context(tc.tile_pool(name="pa2", bufs=2, space="PSUM"))
                for jc in range(4):
                    acc = pa2.tile([P, K], F32, tag="kpj")
                    for dd in range(8):
                        nc.tensor.matmul(
                            acc[:],
                            lhsT=wk_sb[:, dd * DG + jc * P: dd * DG + (jc + 1) * P],
                            rhs=xcxv_sb[:, dd * 2 * K: dd * 2 * K + K],
                            start=(dd == 0), stop=(dd == 7))
                    nc.scalar.copy(out=kprojT_sb[:, jc * K:(jc + 1) * K], in_=acc[:])
                for fc in range(2):
                    acc2 = pa2.tile([P, DG], F32, tag="vpj")
                    for dd in range(8):
                        nc.tensor.matmul(
                            acc2[:],
                            lhsT=xcxv_sb[:, dd * 2 * K + K + fc * P: dd * 2 * K + K + (fc + 1) * P],
                            rhs=wv_sb[:, dd * DG:(dd + 1) * DG],
                            start=(dd == 0), stop=(dd == 7))
                    nc.scalar.copy(out=vproj_sb[:, fc * DG:(fc + 1) * DG], in_=acc2[:])

            # ---------------- Phase B: qT ----------------
            with ExitStack() as bctx:
                wqp = bctx.enter_context(tc.tile_pool(name="wqp", bufs=1))
                xtb = bctx.enter_context(tc.tile_pool(name="xtb", bufs=2))
                pq = bctx.enter_context(tc.tile_pool(name="pq", bufs=3, space="PSUM"))

                wq_sb = wqp.tile([P, 8 * DG], MMDT, tag="wq")
                for dd in range(8):
                    nc.sync.dma_start(out=wq_sb[:, dd * DG:(dd + 1) * DG],
                                      in_=wq_d[dd * P:(dd + 1) * P, :])

                for nb in range(NB):
                    xT_t = xtb.tile([P, 8 * DG], MMDT, tag="xT")
                    # xbar DMA transpose: x[nb block, dd chunk] -> xT tile
                    for dd in range(8):
                        nc.sync.dma_start_transpose(
                            out=xT_t[:, dd * DG:(dd + 1) * DG],
                            in_=x_d[nb * DG:(nb + 1) * DG, dd * P:(dd + 1) * P])
                    for jc in range(4):
                        accq = pq.tile([P, DG], F32, tag="pq")
                        for dd in range(8):
                            nc.tensor.matmul(
                                accq[:],
                                lhsT=wq_sb[:, dd * DG + jc * P: dd * DG + (jc + 1) * P],
                                rhs=xT_t[:, dd * DG:(dd + 1) * DG],
                                start=(dd == 0), stop=(dd == 7))
                        nc.vector.tensor_copy(
                            qT_sb[:, jc * N + nb * DG: jc * N + (nb + 1) * DG],
                            accq[:])

            # ---------------- Phase C/D/E fused per n-block ----------------
            with ExitStack() as cctx:
                pex = cctx.enter_context(tc.tile_pool(name="pex", bufs=2))
                osb = cctx.enter_context(tc.tile_pool(name="osb", bufs=2))
                otp = cctx.enter_context(tc.tile_pool(name="otp", bufs=3))
                outp = cctx.enter_context(tc.tile_pool(name="outp", bufs=3))
                pc = cctx.enter_context(tc.tile_pool(name="pc", bufs=7, space="PSUM"))

                for nb in range(NB):
                    o_tiles = [osb.tile([P, DG], MMDT, tag=f"o{nn2}", name=f"o{nn2}") for nn2 in range(4)]
                    pexp_tiles = {}
                    s4_tiles = {}
                    # scores + exp + sums, half the heads at a time so the
                    # stats transpose can free pexp slots sooner
                    for hg in range(2):
                        for hh in range(hg * 4, hg * 4 + 4):
                            jc, p0 = hh // 2, (hh % 2) * DH
                            pexp = pex.tile([P, 2 * DG], MMDT, tag=f"pexp{hh % 4}")
                            pexp_tiles[hh] = pexp
                            m = hh % 4
                            if m == 0:
                                s4 = pc.tile([4, DG], F32, tag="c", name="s4")
                                s4_tiles[hg] = s4
                            s4 = s4_tiles[hg]
                            for fc in range(2):
                                at = pc.tile([P, DG], F32, tag="c")
                                nc.tensor.matmul(
                                    at[:],
                                    lhsT=kprojT_sb[p0:p0 + DH, jc * K + fc * P: jc * K + (fc + 1) * P],
                                    rhs=qT_sb[p0:p0 + DH, jc * N + nb * DG: jc * N + (nb + 1) * DG],
                                    start=True, stop=True)
                                nc.scalar.activation(pexp[:, fc * DG:(fc + 1) * DG], at[:],
                                                     mybir.ActivationFunctionType.Exp)
                                nc.tensor.matmul(
                                    s4[:], lhsT=sel_sb[:, m * 4:(m + 1) * 4],
                                    rhs=pexp[:, fc * DG:(fc + 1) * DG],
                                    start=(m == 0 and fc == 0),
                                    stop=(m == 3 and fc == 1))
                        nc.vector.tensor_copy(
                            sums_halves[hg][0:4, nb * DG:(nb + 1) * DG],
                            s4_tiles[hg][:])
                        # stats for this half-batch of heads
                        h0 = hg * 4
                        for nn2 in range(4):
                            ci = nb * 4 + nn2
                            st = pc.tile([P, HL], F32, tag="c")
                            nc.tensor.transpose(
                                st[:, :4],
                                sums_halves[hg][0:4, ci * P:(ci + 1) * P],
                                id_f32[:4, :4])
                            nc.vector.reciprocal(
                                recips_sb[:, ci * HL + h0: ci * HL + h0 + 4],
                                st[:, :4])
                        # attention output for this half-batch
                        for hh in range(hg * 4, hg * 4 + 4):
                            pexp = pexp_tiles[hh]
                            for nn2 in range(4):
                                ci = nb * 4 + nn2
                                po = pc.tile([P, DH], F32, tag="c")
                                for fc in range(2):
                                    nc.tensor.matmul(
                                        po[:],
                                        lhsT=pexp[:, fc * DG + nn2 * P: fc * DG + (nn2 + 1) * P],
                                        rhs=vproj_sb[:, fc * DG + hh * DH: fc * DG + (hh + 1) * DH],
                                        start=(fc == 0), stop=(fc == 1))
                                nc.vector.tensor_scalar_mul(
                                    o_tiles[nn2][:, hh * DH:(hh + 1) * DH], po[:],
                                    recips_sb[:, ci * HL + hh: ci * HL + hh + 1])
                    # Phase E for this n-block
                    for nn2 in range(4):
                        ci = nb * 4 + nn2
                        ot = otp.tile([P, DG], MMDT, tag="ot")
                        for jc2 in range(4):
                            nc.sync.dma_start_transpose(
                                out=ot[:, jc2 * P:(jc2 + 1) * P],
                                in_=o_tiles[nn2][:, jc2 * P:(jc2 + 1) * P])
                        outsb = outp.tile([P, D], F32, tag="outsb")
                        for half in range(2):
                            pe_acc = pc.tile([P, DG], F32, tag="c")
                            for jc2 in range(4):
                                nc.tensor.matmul(
                                    pe_acc[:],
                                    lhsT=ot[:, jc2 * P:(jc2 + 1) * P],
                                    rhs=wproj_sb[:, jc2 * D + half * DG: jc2 * D + (half + 1) * DG],
                                    start=(jc2 == 0), stop=(jc2 == 3))
                            nc.vector.tensor_copy(outsb[:, half * DG:(half + 1) * DG],
                                                  pe_acc[:])
                        nc.sync.dma_start(out=out_d[ci * P:(ci + 1) * P, :], in_=outsb[:])
    nc.compile()
    return nc


def _np_mm(a):
    return np.ascontiguousarray(np.asarray(a), dtype=mybir.dt.np(MMDT))


def kernel(x, Wq, Wkv, Wproj, bproj, proj_k, proj_v):
    x = np.asarray(x)
    Wq, Wkv, Wproj = np.asarray(Wq), np.asarray(Wkv), np.asarray(Wproj)
    bproj, proj_k, proj_v = np.asarray(bproj), np.asarray(proj_k), np.asarray(proj_v)

    if "nc" not in _cache:
        _cache["nc"] = build_nc()
    nc = _cache["nc"]

    scale = np.float32(DH ** -0.5)
    projkv = _np_mm(np.concatenate([proj_k, proj_v], axis=1))
    in_maps = []
    for c in range(8):
        b, g = c // 2, c % 2
        cols = slice(g * DG, (g + 1) * DG)
        in_maps.append({
            "x": _np_mm(x[b]),
            "projkv": projkv,
            "wq": _np_mm(scale * Wq[:, cols]),
            "wk": _np_mm(Wkv[:, :D][:, cols]),
            "wv": _np_mm(Wkv[:, D:][:, cols]),
            "wproj": _np_mm(Wproj[cols, :]),
        })
    res = run_bass_kernel_spmd(nc, in_maps, list(range(8)),
                               trace=bool(os.environ.get("LINF_TRACE")))
    _cache["last_result"] = res
    outs = [r["out"] for r in res.results]
    full = np.stack([outs[2 * b] + outs[2 * b + 1] for b in range(4)])
    full = full + np.asarray(bproj, np.float32)
    return full.astype(np.float32)


# revision 16
# speedup vs baseline: 1.4195x; 1.4195x over previous
"""Linformer self-attention on 8 Trainium2 NeuronCores.

Problem (hardcoded shapes): x [4,4096,1024] f32; per batch:
  q = scale*(x@Wq); kv = x@Wkv; keys/values compressed 4096->256 via
  proj_k/proj_v; 16-head attention (dh=64, k=256); out @ Wproj + bproj.

Sharding: 8 cores = 4 batches x 2 head-groups (8 heads / 512 cols each).
Each core computes a partial [4096,1024] output (Wproj row-split); host
sums the pair and adds bias.

Per-core dataflow (all matmuls use out = lhsT.T @ rhs, K<=128 partitions):
  A : xcxvT[1024,512] = x.T @ [proj_k|proj_v]          (contract n, x natural lhsT)
  A2: kprojT[512,256] = Wk_g.T @ xcT ; vproj[256,512] = xvT.T @ Wv_g
  B : qT[512,4096] = Wq_g.T @ x.T    (x.T loaded via xbar DMA transpose)
  C : per head, per n-block: AT[256,n] = kprojT_h.T @ qT_h  -> exp (ACT, no
      max-subtract: |scores| < ~6) -> column sums via ones-matmul ->
      transpose-stats -> reciprocal
  D : O[n,64] = PexpT.T @ vproj_h, normalized by recip at PSUM eviction
  E : out[n,1024] = O.T-transpose @ Wproj_g
"""

import os
import numpy as np

import concourse.bass as bass
import concourse.mybir as mybir
import concourse.tile as tile
from concourse import bacc
from concourse.bass_utils import run_bass_kernel_spmd
from concourse.masks import make_identity

P = 128
N, D, K, DG, DH = 4096, 1024, 256, 512, 64
NCHUNKS = N // P          # 32 chunks of 128 rows
NB = 8                    # n-blocks of 512
HL = 8                    # heads per core
F32 = mybir.dt.float32

# matmul operand dtype. bf16 runs at 1 cycle/row on the PE (fp32 is 4x
# slower); float32r is rejected by this walrus build (producers cannot
# round to fp32r). Accuracy vs the f32 reference: ~6e-3 max-rel.
MMDT_NAME = os.environ.get("LINF_MMDT", "bfloat16")
MMDT = getattr(mybir.dt, MMDT_NAME)

_cache = {}


def build_nc():
    nc = bacc.Bacc(None, target_bir_lowering=False, debug=False)

    x_d = nc.dram_tensor("x", [N, D], MMDT, kind="ExternalInput")
    pkv_d = nc.dram_tensor("projkv", [N, 2 * K], MMDT, kind="ExternalInput")
    wq_d = nc.dram_tensor("wq", [D, DG], MMDT, kind="ExternalInput")
    wk_d = nc.dram_tensor("wk", [D, DG], MMDT, kind="ExternalInput")
    wv_d = nc.dram_tensor("wv", [D, DG], MMDT, kind="ExternalInput")
    wp_d = nc.dram_tensor("wproj", [DG, D], MMDT, kind="ExternalInput")
    out_d = nc.dram_tensor("out", [N, D], F32, kind="ExternalOutput")

    with tile.TileContext(nc) as tc:
        from contextlib import ExitStack
        with ExitStack() as ctx:
            res = ctx.enter_context(tc.tile_pool(name="res", bufs=1))
            id_mm = res.tile([P, P], MMDT, tag="id_mm")
            make_identity(nc, id_mm[:])
            id_f32 = res.tile([P, P], F32, tag="id_f32")
            make_identity(nc, id_f32[:])
            # sel[:, m*4:m*4+4] is a [128,4] matrix whose column m is all
            # ones: used to matmul-accumulate per-head exp-sums into row m of
            # a [4, 512] PSUM tile (engine partition offsets must be 0/32/64,
            # so rows are written via matmul selectors, not offset copies).
            sel_sb = res.tile([P, 16], MMDT, tag="sel")
            nc.any.memset(sel_sb[:], 0.0)
            for m in range(4):
                nc.any.memset(sel_sb[:, m * 4 + m: m * 4 + m + 1], 1.0)

            wproj_sb = res.tile([P, 4 * D], MMDT, tag="wproj")
            for jc in range(4):
                nc.sync.dma_start(out=wproj_sb[:, jc * D:(jc + 1) * D],
                                  in_=wp_d[jc * P:(jc + 1) * P, :])
            kprojT_sb = res.tile([P, 4 * K], MMDT, tag="kprojT")
            vproj_sb = res.tile([P, 2 * DG], MMDT, tag="vproj")
            qT_sb = res.tile([P, 4 * N], MMDT, tag="qT")
            sums_sb0 = res.tile([4, N], F32, tag="sums0")
            sums_sb1 = res.tile([4, N], F32, tag="sums1")
            sums_halves = [sums_sb0, sums_sb1]
            recips_sb = res.tile([P, NCHUNKS * HL], F32, tag="recips")

            # ---------------- Phase A + A2 ----------------
            with ExitStack() as actx:
                wkvp = actx.enter_context(tc.tile_pool(name="wkv", bufs=1))
                xcxvp = actx.enter_context(tc.tile_pool(name="xcxv", bufs=1))
                xin = actx.enter_context(tc.tile_pool(name="xin", bufs=3))

                wk_sb = wkvp.tile([P, 8 * DG], MMDT, tag="wk")
                wv_sb = wkvp.tile([P, 8 * DG], MMDT, tag="wv")
                for dd in range(8):
                    nc.sync.dma_start(out=wk_sb[:, dd * DG:(dd + 1) * DG],
                                      in_=wk_d[dd * P:(dd + 1) * P, :])
                    nc.sync.dma_start(out=wv_sb[:, dd * DG:(dd + 1) * DG],
                                      in_=wv_d[dd * P:(dd + 1) * P, :])
                xcxv_sb = xcxvp.tile([P, 8 * 2 * K], MMDT, tag="xcxv")

                with tc.tile_pool(name="pa", bufs=1, space="PSUM") as pa:
                    accs = [pa.tile([P, 2 * K], F32, tag=f"pa{dd}", name=f"pa{dd}") for dd in range(8)]
                    for nn in range(NCHUNKS):
                        x_t = xin.tile([P, D], MMDT, tag="x_t")
                        kv_t = xin.tile([P, 2 * K], MMDT, tag="kv_t")
                        nc.sync.dma_start(out=x_t[:], in_=x_d[nn * P:(nn + 1) * P, :])
                        nc.sync.dma_start(out=kv_t[:], in_=pkv_d[nn * P:(nn + 1) * P, :])
                        for dd in range(8):
                            nc.tensor.matmul(accs[dd][:],
                                             lhsT=x_t[:, dd * P:(dd + 1) * P],
                                             rhs=kv_t[:],
                                             start=(nn == 0), stop=(nn == NCHUNKS - 1))
                    for dd in range(8):
                        nc.scalar.copy(out=xcxv_sb[:, dd * 2 * K:(dd + 1) * 2 * K],
                                       in_=accs[dd][:])

                pa2 = actx.enter_context(tc.tile_pool(name="pa2", bufs=2, space="PSUM"))
                for jc in range(4):
                    acc = pa2.tile([P, K], F32, tag="kpj")
                    for dd in range(8):
                        nc.tensor.matmul(
                            acc[:],
                            lhsT=wk_sb[:, dd * DG + jc * P: dd * DG + (jc + 1) * P],
                            rhs=xcxv_sb[:, dd * 2 * K: dd * 2 * K + K],
                            start=(dd == 0), stop=(dd == 7))
                    nc.scalar.copy(out=kprojT_sb[:, jc * K:(jc + 1) * K], in_=acc[:])
                for fc in range(2):
                    acc2 = pa2.tile([P, DG], F32, tag="vpj")
                    for dd in range(8):
                        nc.tensor.matmul(
                            acc2[:],
                            lhsT=xcxv_sb[:, dd * 2 * K + K + fc * P: dd * 2 * K + K + (fc + 1) * P],
                            rhs=wv_sb[:, dd * DG:(dd + 1) * DG],
                            start=(dd == 0), stop=(dd == 7))
                    nc.scalar.copy(out=vproj_sb[:, fc * DG:(fc + 1) * DG], in_=acc2[:])

            # ---------------- Phase B: qT ----------------
            with ExitStack() as bctx:
                wqp = bctx.enter_context(tc.tile_pool(name="wqp", bufs=1))
                xtb = bctx.enter_context(tc.tile_pool(name="xtb", bufs=2))
                pq = bctx.enter_context(tc.tile_pool(name="pq", bufs=3, space="PSUM"))

                wq_sb = wqp.tile([P, 8 * DG], MMDT, tag="wq")
                for dd in range(8):
                    nc.sync.dma_start(out=wq_sb[:, dd * DG:(dd + 1) * DG],
                                      in_=wq_d[dd * P:(dd + 1) * P, :])

                for nb in range(NB):
                    xT_t = xtb.tile([P, 8 * DG], MMDT, tag="xT")
                    # xbar DMA transpose: x[nb block, dd chunk] -> xT tile
                    for dd in range(8):
                        nc.sync.dma_start_transpose(
                            out=xT_t[:, dd * DG:(dd + 1) * DG],
                            in_=x_d[nb * DG:(nb + 1) * DG, dd * P:(dd + 1) * P])
                    for jc in range(4):
                        accq = pq.tile([P, DG], F32, tag="pq")
                        for dd in range(8):
                            nc.tensor.matmul(
                                accq[:],
                                lhsT=wq_sb[:, dd * DG + jc * P: dd * DG + (jc + 1) * P],
                                rhs=xT_t[:, dd * DG:(dd + 1) * DG],
                                start=(dd == 0), stop=(dd == 7))
                        nc.vector.tensor_copy(
                            qT_sb[:, jc * N + nb * DG: jc * N + (nb + 1) * DG],
                            accq[:])

            # ---------------- Phase C/D/E fused per n-block ----------------
            with ExitStack() as cctx:
                pex = cctx.enter_context(tc.tile_pool(name="pex", bufs=2))
                osb = cctx.enter_context(tc.tile_pool(name="osb", bufs=2))
                otp = cctx.enter_context(tc.tile_pool(name="otp", bufs=3))
                outp = cctx.enter_context(tc.tile_pool(name="outp", bufs=3))
                pc = cctx.enter_context(tc.tile_pool(name="pc", bufs=7, space="PSUM"))

                for nb in range(NB):
                    o_tiles = [osb.tile([P, DG], MMDT, tag=f"o{nn2}", name=f"o{nn2}") for nn2 in range(4)]
                    pexp_tiles = {}
                    s4_tiles = {}
                    # scores + exp + sums, half the heads at a time so the
                    # stats transpose can free pexp slots sooner
                    for hg in range(2):
                        for hh in range(hg * 4, hg * 4 + 4):
                            jc, p0 = hh // 2, (hh % 2) * DH
                            pexp = pex.tile([P, 2 * DG], MMDT, tag=f"pexp{hh % 4}")
                            pexp_tiles[hh] = pexp
                            m = hh % 4
                            if m == 0:
                                s4 = pc.tile([4, DG], F32, tag="c", name="s4")
                                s4_tiles[hg] = s4
                            s4 = s4_tiles[hg]
                            for fc in range(2):
                                at = pc.tile([P, DG], F32, tag="c")
                                nc.tensor.matmul(
                                    at[:],
                                    lhsT=kprojT_sb[p0:p0 + DH, jc * K + fc * P: jc * K + (fc + 1) * P],
                                    rhs=qT_sb[p0:p0 + DH, jc * N + nb * DG: jc * N + (nb + 1) * DG],
                                    start=True, stop=True)
                                nc.scalar.activation(pexp[:, fc * DG:(fc + 1) * DG], at[:],
                                                     mybir.ActivationFunctionType.Exp)
                                nc.tensor.matmul(
                                    s4[:], lhsT=sel_sb[:, m * 4:(m + 1) * 4],
                                    rhs=pexp[:, fc * DG:(fc + 1) * DG],
                                    start=(m == 0 and fc == 0),
                                    stop=(m == 3 and fc == 1))
                        nc.vector.tensor_copy(
                            sums_halves[hg][0:4, nb * DG:(nb + 1) * DG],
                            s4_tiles[hg][:])
                        # stats for this half-batch of heads
                        h0 = hg * 4
                        for nn2 in range(4):
                            ci = nb * 4 + nn2
                            st = pc.tile([P, HL], F32, tag="c")
                            nc.tensor.transpose(
                                st[:, :4],
                                sums_halves[hg][0:4, ci * P:(ci + 1) * P],
                                id_f32[:4, :4])
                            nc.vector.reciprocal(
                                recips_sb[:, ci * HL + h0: ci * HL + h0 + 4],
                                st[:, :4])
                        # attention output for this half-batch
                        for hh in range(hg * 4, hg * 4 + 4):
                            pexp = pexp_tiles[hh]
                            for nn2 in range(4):
                                ci = nb * 4 + nn2
                                po = pc.tile([P, DH], F32, tag="c")
                                for fc in range(2):
                                    nc.tensor.matmul(
                                        po[:],
                                        lhsT=pexp[:, fc * DG + nn2 * P: fc * DG + (nn2 + 1) * P],
                                        rhs=vproj_sb[:, fc * DG + hh * DH: fc * DG + (hh + 1) * DH],
                                        start=(fc == 0), stop=(fc == 1))
                                nc.vector.tensor_scalar_mul(
                                    o_tiles[nn2][:, hh * DH:(hh + 1) * DH], po[:],
                                    recips_sb[:, ci * HL + hh: ci * HL + hh + 1])
                    # Phase E for this n-block
                    for nn2 in range(4):
                        ci = nb * 4 + nn2
                        ot = otp.tile([P, DG], MMDT, tag="ot")
                        for jc2 in range(4):
                            nc.sync.dma_start_transpose(
                                out=ot[:, jc2 * P:(jc2 + 1) * P],
                                in_=o_tiles[nn2][:, jc2 * P:(jc2 + 1) * P])
                        outsb = outp.tile([P, D], F32, tag="outsb")
                        for half in range(2):
                            pe_acc = pc.tile([P, DG], F32, tag="c")
                            for jc2 in range(4):
                                nc.tensor.matmul(
                                    pe_acc[:],
                                    lhsT=ot[:, jc2 * P:(jc2 + 1) * P],
                                    rhs=wproj_sb[:, jc2 * D + half * DG: jc2 * D + (half + 1) * DG],
                                    start=(jc2 == 0), stop=(jc2 == 3))
                            nc.vector.tensor_copy(outsb[:, half * DG:(half + 1) * DG],
                                                  pe_acc[:])
                        nc.sync.dma_start(out=out_d[ci * P:(ci + 1) * P, :], in_=outsb[:])
    nc.compile()
    return nc


def _np_mm(a):
    return np.ascontiguousarray(np.asarray(a), dtype=mybir.dt.np(MMDT))


def kernel(x, Wq, Wkv, Wproj, bproj, proj_k, proj_v):
    x = np.asarray(x)
    Wq, Wkv, Wproj = np.asarray(Wq), np.asarray(Wkv), np.asarray(Wproj)
    bproj, proj_k, proj_v = np.asarray(bproj), np.asarray(proj_k), np.asarray(proj_v)

    if "nc" not in _cache:
        _cache["nc"] = build_nc()
    nc = _cache["nc"]

    scale = np.float32(DH ** -0.5)
    projkv = _np_mm(np.concatenate([proj_k, proj_v], axis=1))
    in_maps = []
    for c in range(8):
        b, g = c // 2, c % 2
        cols = slice(g * DG, (g + 1) * DG)
        in_maps.append({
            "x": _np_mm(x[b]),
            "projkv": projkv,
            "wq": _np_mm(scale * Wq[:, cols]),
            "wk": _np_mm(Wkv[:, :D][:, cols]),
            "wv": _np_mm(Wkv[:, D:][:, cols]),
            "wproj": _np_mm(Wproj[cols, :]),
        })
    res = run_bass_kernel_spmd(nc, in_maps, list(range(8)),
                               trace=bool(os.environ.get("LINF_TRACE")))
    _cache["last_result"] = res
    outs = [r["out"] for r in res.results]
    full = np.stack([outs[2 * b] + outs[2 * b + 1] for b in range(4)])
    full = full + np.asarray(bproj, np.float32)
    return full.astype(np.float32)
